# revision 8
# baseline (speedup 1.0000x reference)
"""DGL capsule routing layer on 8 trn2 NeuronCores (Bass/Tile).

Math: for routing_num iterations,
    c = softmax(b, axis=out)                        # b0 = 0
    s = einsum('io,iof->of', c, uh)
    v = squash(s)
    b = b + einsum('iof,of->io', uh, v)
Output: final v [OUT, F].

Key identity: b_t = uh . (v_1 + ... + v_t)  (b is linear in uh), so b is
never materialized across iterations; each iteration is one streaming pass
over uh with w_t = cumulative sum of v's:
    pass t: b = sum_f uh[i,o,f]*w[o,f]; e = exp(b); r_i = 1/sum_o e
            s[o,f] = sum_i r_i * e[i,o] * uh[i,o,f]   (partial per core)
            AllReduce(s); v = squash(s); w += v
Pass 1 has c uniform (=1/OUT) so it is a plain column-sum on the PE.

Sharding: i (in_nodes) split across 8 cores, 512 rows each. The per-pass
s-partial [1024,16] is AllReduced (64KB, ~10us).
"""

import numpy as np
from contextlib import ExitStack

import concourse.bass as bass
import concourse.mybir as mybir
import concourse.tile as tile
from concourse import bacc
from concourse import bass_utils
from concourse.masks import make_identity

F32 = mybir.dt.float32
AX = mybir.AxisListType
AF = mybir.ActivationFunctionType

IN_NODES, OUT_NODES, F_SIZE = 4096, 1024, 16
CORES = 8
I_LOC = IN_NODES // CORES          # 512 in-nodes per core
ROW = OUT_NODES * F_SIZE           # 16384 floats per in-node row
P = 128
NBLK = I_LOC // P                  # 4 i-blocks per core
HALF = ROW // 2                    # 8192 (o-half of a row block)
CH = 2048                          # elementwise work chunk (elems/partition)
NCH_H = HALF // CH                 # 4 chunks per half
MM_W = P                           # 128-wide matmul weight chunks
NCOL = ROW // MM_W                 # 128 psum columns holding s


def _body(nc, tc, uh, v_out, R, rg):
    uh_t = uh.rearrange("(n p) r -> n p r", p=P)   # [NBLK, 128, 16384]

    with ExitStack() as ctx:
        io = ctx.enter_context(tc.tile_pool(name="io", bufs=2))
        work = ctx.enter_context(tc.tile_pool(name="work", bufs=2))
        small = ctx.enter_context(tc.tile_pool(name="small", bufs=2))
        persist = ctx.enter_context(tc.tile_pool(name="persist", bufs=1))
        pschunk = ctx.enter_context(tc.tile_pool(name="pschunk", bufs=2, space="PSUM"))
        psacc = ctx.enter_context(tc.tile_pool(name="psacc", bufs=2, space="PSUM"))
        dram = ctx.enter_context(tc.tile_pool(name="dram", bufs=2, space="DRAM"))

        # constants
        c0 = persist.tile([P, 1], F32, name="c0")
        nc.vector.memset(c0, 1.0 / OUT_NODES)
        ident = persist.tile([P, P], F32, name="ident")
        make_identity(nc, ident)
        w_sb = w_acc = None
        if R > 1:
            w_sb = persist.tile([P, ROW], F32, name="w_sb")
            w_acc = persist.tile([P, P], F32, name="w_acc")

        for t in range(1, R + 1):
            s_acc = small.tile([P, NCOL], F32, tag="s_acc")
            for blk in range(NBLK):
                s_ps = psacc.tile([P, NCOL], F32, tag="s_ps")
                uts = []
                for h in range(2):
                    ut = io.tile([P, HALF], F32, tag="ut")
                    nc.sync.dma_start(ut, uh_t[blk, :, h * HALF:(h + 1) * HALF])
                    uts.append(ut)
                if t == 1:
                    # uniform c: s = sum_i uh/OUT. lhsT = uh chunk, rhs = 1/OUT.
                    for h in range(2):
                        for c in range(HALF // MM_W):
                            g = h * (HALF // MM_W) + c
                            nc.tensor.matmul(
                                s_ps[:, g:g + 1],
                                uts[h][:, c * MM_W:(c + 1) * MM_W],
                                c0,
                                start=True, stop=True,
                                skip_group_check=True,
                            )
                else:
                    b = small.tile([P, OUT_NODES], F32, tag="b")
                    for h in range(2):
                        for k in range(NCH_H):
                            sl = slice(k * CH, (k + 1) * CH)
                            g0 = h * HALF + k * CH
                            tm = work.tile([P, CH], F32, tag="tm")
                            # b-mul on GpSimd: runs concurrently with DVE
                            # (tensor_tensor/reduce never take the shared port)
                            nc.gpsimd.tensor_mul(
                                tm, uts[h][:, sl], w_sb[:, g0:g0 + CH])
                            o0 = g0 // F_SIZE
                            nc.vector.reduce_sum(
                                b[:, o0:o0 + CH // F_SIZE],
                                tm.rearrange("p (o f) -> p o f", f=F_SIZE),
                                axis=AX.X,
                            )
                    e = small.tile([P, OUT_NODES], F32, tag="e")
                    den = small.tile([P, 1], F32, tag="den")
                    nc.scalar.activation(e, b, AF.Exp, accum_out=den)
                    rinv = small.tile([P, 1], F32, tag="rinv")
                    nc.vector.reciprocal(rinv, den)
                    for h in range(2):
                        for k in range(NCH_H):
                            sl = slice(k * CH, (k + 1) * CH)
                            g0 = h * HALF + k * CH
                            o0 = g0 // F_SIZE
                            och = CH // F_SIZE
                            pt = work.tile([P, CH], F32, tag="tm")
                            nc.vector.tensor_mul(
                                pt.rearrange("p (o f) -> p o f", f=F_SIZE),
                                uts[h][:, sl].rearrange("p (o f) -> p o f", f=F_SIZE),
                                e[:, o0:o0 + och][:, :, None].broadcast_to(
                                    [P, och, F_SIZE]),
                            )
                            for c in range(CH // MM_W):
                                g = g0 // MM_W + c
                                nc.tensor.matmul(
                                    s_ps[:, g:g + 1],
                                    pt[:, c * MM_W:(c + 1) * MM_W],
                                    rinv,
                                    start=True, stop=True,
                                    skip_group_check=True,
                                )
                # fold this block's partial into the SBUF accumulator
                if blk == 0:
                    nc.vector.tensor_copy(s_acc, s_ps)
                else:
                    nc.vector.tensor_add(s_acc, s_acc, s_ps)
            # s_acc[m, g] = s_flat[g*128 + m] (this core's partial)
            ar_in = dram.tile([P, NCOL], F32, tag="ar_in")
            nc.sync.dma_start(ar_in, s_acc)
            ar_out = dram.tile([P, NCOL], F32, tag="ar_out")
            nc.gpsimd.collective_compute(
                "AllReduce", mybir.AluOpType.add, replica_groups=rg,
                ins=[ar_in.opt()], outs=[ar_out.opt()],
            )
            sT = small.tile([P, NCOL], F32, tag="sT")
            nc.sync.dma_start(sT, ar_out)
            # transpose -> s2[p, (j,f)] with o = p*8 + j
            ps_t = pschunk.tile([P, P], F32, tag="ps_t")
            nc.tensor.transpose(ps_t, sT, ident)
            s2 = small.tile([P, P], F32, tag="s2")
            nc.scalar.copy(s2, ps_t)
            # squash: v = s * sqrt(sq)/(1+sq), sq = sum_f s^2
            ssq = small.tile([P, P], F32, tag="ssq")
            nc.vector.tensor_mul(ssq, s2, s2)
            sq = small.tile([P, 8], F32, tag="sq")
            nc.vector.reduce_sum(
                sq, ssq.rearrange("p (j f) -> p j f", f=F_SIZE), axis=AX.X)
            y = small.tile([P, 8], F32, tag="y")
            nc.scalar.sqrt(y, sq)
            # one Newton step: y <- 0.5*(y + sq/y) (ACT sqrt table is loose)
            ry = small.tile([P, 8], F32, tag="ry")
            nc.vector.reciprocal(ry, y)
            t1 = small.tile([P, 8], F32, tag="t1")
            nc.vector.tensor_mul(t1, sq, ry)
            nc.vector.tensor_add(t1, t1, y)
            nc.vector.tensor_scalar_mul(t1, t1, 0.5)
            d1 = small.tile([P, 8], F32, tag="d1")
            nc.vector.tensor_scalar_add(d1, sq, 1.0)
            rd = small.tile([P, 8], F32, tag="rd")
            nc.vector.reciprocal(rd, d1)
            sc = small.tile([P, 8], F32, tag="sc")
            nc.vector.tensor_mul(sc, t1, rd)
            v_sb = small.tile([P, P], F32, tag="v_sb")
            nc.vector.tensor_mul(
                v_sb.rearrange("p (j f) -> p j f", f=F_SIZE),
                s2.rearrange("p (j f) -> p j f", f=F_SIZE),
                sc[:, :, None].broadcast_to([P, 8, F_SIZE]),
            )
            if t == R:
                nc.sync.dma_start(
                    v_out.rearrange("(p j) f -> p (j f)", j=8), v_sb)
            else:
                if t == 1:
                    nc.scalar.copy(w_acc, v_sb)
                else:
                    nc.vector.tensor_add(w_acc, w_acc, v_sb)
                # broadcast w to all partitions: column p0 of w_sb's 128-wide
                # chunk = w_acc row p0 replicated; lhsT = identity col p0
                # broadcast over free dim selects that row on the PE.
                for c in range(ROW // 512):
                    pbc = pschunk.tile([P, 512], F32, tag="pbc")
                    for q in range(4):
                        p0 = 4 * c + q
                        nc.tensor.matmul(
                            pbc[:, q * P:(q + 1) * P],
                            ident[:, p0:p0 + 1].broadcast_to([P, P]),
                            w_acc, start=True, stop=True,
                            skip_group_check=True)
                    nc.scalar.copy(w_sb[:, c * 512:(c + 1) * 512], pbc)


def _build(routing_num: int):
    R = int(routing_num)
    assert R >= 1
    nc = bacc.Bacc(
        "TRN2", target_bir_lowering=False, debug=False, num_devices=CORES)
    uh = nc.dram_tensor("uh", [I_LOC, ROW], F32, kind="ExternalInput")
    v_out = nc.dram_tensor("v_out", [OUT_NODES, F_SIZE], F32,
                           kind="ExternalOutput")
    rg = [list(range(CORES))]
    with tile.TileContext(nc) as tc:
        _body(nc, tc, uh.ap(), v_out.ap(), R, rg)
    nc.compile()
    return nc


_CACHE: dict = {}


def _get_nc(routing_num: int):
    R = int(routing_num)
    if R not in _CACHE:
        _CACHE[R] = _build(R)
    return _CACHE[R]


def _shard(u_hat: np.ndarray):
    uh = np.ascontiguousarray(np.asarray(u_hat, dtype=np.float32))
    assert uh.shape == (IN_NODES * OUT_NODES, F_SIZE), uh.shape
    uh = uh.reshape(IN_NODES, ROW)
    return [
        {"uh": np.ascontiguousarray(uh[k * I_LOC:(k + 1) * I_LOC])}
        for k in range(CORES)
    ]


def run(u_hat, routing_num, trace=False):
    nc = _get_nc(routing_num)
    in_maps = _shard(u_hat)
    res = bass_utils.run_bass_kernel_spmd(
        nc, in_maps, core_ids=list(range(CORES)), trace=trace)
    return res


def kernel(u_hat, routing_num):
    res = run(u_hat, routing_num, trace=False)
    return np.asarray(res.results[0]["v_out"], dtype=np.float32)


# revision 9
# speedup vs baseline: 1.1607x; 1.1607x over previous
"""DGL capsule routing layer on 8 trn2 NeuronCores (Bass/Tile).

Math: for routing_num iterations,
    c = softmax(b, axis=out)                        # b0 = 0
    s = einsum('io,iof->of', c, uh)
    v = squash(s)
    b = b + einsum('iof,of->io', uh, v)
Output: final v [OUT, F].

Key identity: b_t = uh . (v_1 + ... + v_t)  (b is linear in uh), so b is
never materialized across iterations; each iteration is one streaming pass
over uh with w_t = cumulative sum of v's:
    pass t: b = sum_f uh[i,o,f]*w[o,f]; e = exp(b); r_i = 1/sum_o e
            s[o,f] = sum_i r_i * e[i,o] * uh[i,o,f]   (partial per core)
            AllReduce(s); v = squash(s); w += v
Pass 1 has c uniform (=1/OUT) so it is a pure PE pass.

Sharding: i (in_nodes) split across 8 cores, 512 rows each (4 blocks of
128 partitions). Engine plan per 2048-wide o-f chunk (passes >= 2):
  GpSimd: tm = uh * w_bcast        (2-input mul; DVE TT never contends)
  DVE:    b-slice = segsum_f(tm);  p = e * uh (e broadcast over f)
  ACT:    e = exp(b) with fused denominator accum; psum flushes
  PE:     s-partial = sum_i rinv[i]*p[i,:] as 4x N=512 matmuls with
          rinv as the 1-column stationary operand -> psum [1,2048]
The per-block s partials go straight to DRAM [4,16384]; the AllReduce sums
over cores, and the cheap cross-block sum happens after the AR in the
partition-spread [128,128] layout (3 DVE adds).
"""

import numpy as np
from contextlib import ExitStack

import concourse.bass as bass
import concourse.mybir as mybir
import concourse.tile as tile
from concourse import bacc
from concourse import bass_utils

F32 = mybir.dt.float32
AX = mybir.AxisListType
AF = mybir.ActivationFunctionType

IN_NODES, OUT_NODES, F_SIZE = 4096, 1024, 16
CORES = 8
I_LOC = IN_NODES // CORES          # 512 in-nodes per core
ROW = OUT_NODES * F_SIZE           # 16384 floats per in-node row
P = 128
NBLK = I_LOC // P                  # 4 i-blocks per core
HALF = ROW // 2                    # 8192 (o-half of a row block)
CH = 2048                          # chunk/piece width (elems/partition)
NCH_H = HALF // CH                 # 4 chunks per half
NMM = CH // 512                    # 4 matmuls per piece


def _body(nc, tc, uh, v_out, R, rg):
    uh_t = uh.rearrange("(n p) r -> n p r", p=P)   # [NBLK, 128, 16384]

    with ExitStack() as ctx:
        io = ctx.enter_context(tc.tile_pool(name="io", bufs=2))
        work = ctx.enter_context(tc.tile_pool(name="work", bufs=2))
        small = ctx.enter_context(tc.tile_pool(name="small", bufs=2))
        persist = ctx.enter_context(tc.tile_pool(name="persist", bufs=1))
        pspool = ctx.enter_context(tc.tile_pool(name="pspool", bufs=2, space="PSUM"))
        dram = ctx.enter_context(tc.tile_pool(name="dram", bufs=2, space="DRAM"))

        c0 = persist.tile([P, 1], F32, name="c0")
        nc.vector.memset(c0, 1.0 / OUT_NODES)
        w_sb = w_acc = None
        if R > 1:
            w_sb = persist.tile([P, ROW], F32, name="w_sb")
            w_acc = persist.tile([P, P], F32, name="w_acc")

        for t in range(1, R + 1):
            ar_in = dram.tile([NBLK, ROW], F32, tag="ar_in")
            for blk in range(NBLK):
                uts = []
                for h in range(2):
                    ut = io.tile([P, HALF], F32, tag="ut")
                    nc.sync.dma_start(ut, uh_t[blk, :, h * HALF:(h + 1) * HALF])
                    uts.append(ut)
                if t == 1:
                    rinv = c0
                else:
                    b = small.tile([P, OUT_NODES], F32, tag="b")
                    for h in range(2):
                        for k in range(NCH_H):
                            sl = slice(k * CH, (k + 1) * CH)
                            g0 = h * HALF + k * CH
                            tm = work.tile([P, CH], F32, tag="tm")
                            # b-mul on GpSimd (concurrent with DVE TT/reduce)
                            nc.gpsimd.tensor_mul(
                                tm, uts[h][:, sl], w_sb[:, g0:g0 + CH])
                            o0 = g0 // F_SIZE
                            nc.vector.reduce_sum(
                                b[:, o0:o0 + CH // F_SIZE],
                                tm.rearrange("p (o f) -> p o f", f=F_SIZE),
                                axis=AX.X,
                            )
                    e = small.tile([P, OUT_NODES], F32, tag="e")
                    den = small.tile([P, 1], F32, tag="den")
                    nc.scalar.activation(e, b, AF.Exp, accum_out=den)
                    rinv = small.tile([P, 1], F32, tag="rinv")
                    nc.vector.reciprocal(rinv, den)
                for h in range(2):
                    for k in range(NCH_H):
                        sl = slice(k * CH, (k + 1) * CH)
                        g0 = h * HALF + k * CH
                        if t == 1:
                            pt = uts[h][:, sl]
                        else:
                            o0 = g0 // F_SIZE
                            och = CH // F_SIZE
                            pt = work.tile([P, CH], F32, tag="tm")
                            nc.vector.tensor_mul(
                                pt.rearrange("p (o f) -> p o f", f=F_SIZE),
                                uts[h][:, sl].rearrange(
                                    "p (o f) -> p o f", f=F_SIZE),
                                e[:, o0:o0 + och][:, :, None].broadcast_to(
                                    [P, och, F_SIZE]),
                            )
                        ps = pspool.tile([1, CH], F32, tag="ps")
                        for c in range(NMM):
                            nc.tensor.matmul(
                                ps[:, c * 512:(c + 1) * 512],
                                rinv,
                                pt[:, c * 512:(c + 1) * 512],
                                start=True, stop=True,
                                skip_group_check=True,
                            )
                        fl = small.tile([1, CH], F32, tag="fl")
                        nc.scalar.copy(fl, ps)
                        nc.sync.dma_start(ar_in[blk, g0:g0 + CH], fl)
            ar_out = dram.tile([NBLK, ROW], F32, tag="ar_out")
            nc.gpsimd.collective_compute(
                "AllReduce", mybir.AluOpType.add, replica_groups=rg,
                ins=[ar_in.opt()], outs=[ar_out.opt()],
            )
            # s2[p,(j,f)] with o = p*8+j: sum the 4 block rows post-AR
            slds = []
            for blk in range(NBLK):
                sld = small.tile([P, P], F32, tag="sld", bufs=4)
                nc.sync.dma_start(
                    sld, ar_out[blk].rearrange("(p q) -> p q", p=P))
                slds.append(sld)
            s2 = small.tile([P, P], F32, tag="s2")
            nc.vector.tensor_add(s2, slds[0], slds[1])
            nc.vector.tensor_add(s2, s2, slds[2])
            nc.vector.tensor_add(s2, s2, slds[3])
            # squash: v = s * sqrt(sq)/(1+sq), sq = sum_f s^2
            ssq = small.tile([P, P], F32, tag="ssq")
            nc.vector.tensor_mul(ssq, s2, s2)
            sq = small.tile([P, 8], F32, tag="sq")
            nc.vector.reduce_sum(
                sq, ssq.rearrange("p (j f) -> p j f", f=F_SIZE), axis=AX.X)
            y = small.tile([P, 8], F32, tag="y")
            nc.scalar.sqrt(y, sq)
            # one Newton step: y <- 0.5*(y + sq/y) (ACT sqrt table is loose)
            ry = small.tile([P, 8], F32, tag="ry")
            nc.vector.reciprocal(ry, y)
            t1 = small.tile([P, 8], F32, tag="t1")
            nc.vector.tensor_mul(t1, sq, ry)
            nc.vector.tensor_add(t1, t1, y)
            nc.vector.tensor_scalar_mul(t1, t1, 0.5)
            d1 = small.tile([P, 8], F32, tag="d1")
            nc.vector.tensor_scalar_add(d1, sq, 1.0)
            rd = small.tile([P, 8], F32, tag="rd")
            nc.vector.reciprocal(rd, d1)
            sc = small.tile([P, 8], F32, tag="sc")
            nc.vector.tensor_mul(sc, t1, rd)
            v_sb = small.tile([P, P], F32, tag="v_sb")
            nc.vector.tensor_mul(
                v_sb.rearrange("p (j f) -> p j f", f=F_SIZE),
                s2.rearrange("p (j f) -> p j f", f=F_SIZE),
                sc[:, :, None].broadcast_to([P, 8, F_SIZE]),
            )
            if t == R:
                nc.sync.dma_start(
                    v_out.rearrange("(p j) f -> p (j f)", j=8), v_sb)
            else:
                if t == 1:
                    nc.scalar.copy(w_acc, v_sb)
                else:
                    nc.vector.tensor_add(w_acc, w_acc, v_sb)
                # broadcast w to all partitions via DRAM round-trip:
                # w_acc[p,(j,f)] -> flat w_dram[o*16+f] -> [128, ROW] bcast
                w_dram = dram.tile([ROW], F32, tag="w_dram")
                nc.sync.dma_start(
                    w_dram.rearrange("(p q) -> p q", p=P), w_acc)
                wd_b = w_dram.unsqueeze(0)
                for j in range(8):
                    sl = slice(j * CH, (j + 1) * CH)
                    nc.sync.dma_start(
                        w_sb[:, sl],
                        wd_b[:, sl].broadcast_to([P, CH]))


def _build(routing_num: int):
    R = int(routing_num)
    assert R >= 1
    nc = bacc.Bacc(
        "TRN2", target_bir_lowering=False, debug=False, num_devices=CORES)
    uh = nc.dram_tensor("uh", [I_LOC, ROW], F32, kind="ExternalInput")
    v_out = nc.dram_tensor("v_out", [OUT_NODES, F_SIZE], F32,
                           kind="ExternalOutput")
    rg = [list(range(CORES))]
    with tile.TileContext(nc) as tc:
        _body(nc, tc, uh.ap(), v_out.ap(), R, rg)
    nc.compile()
    return nc


_CACHE: dict = {}


def _get_nc(routing_num: int):
    R = int(routing_num)
    if R not in _CACHE:
        _CACHE[R] = _build(R)
    return _CACHE[R]


def _shard(u_hat: np.ndarray):
    uh = np.ascontiguousarray(np.asarray(u_hat, dtype=np.float32))
    assert uh.shape == (IN_NODES * OUT_NODES, F_SIZE), uh.shape
    uh = uh.reshape(IN_NODES, ROW)
    return [
        {"uh": np.ascontiguousarray(uh[k * I_LOC:(k + 1) * I_LOC])}
        for k in range(CORES)
    ]


def run(u_hat, routing_num, trace=False):
    nc = _get_nc(routing_num)
    in_maps = _shard(u_hat)
    res = bass_utils.run_bass_kernel_spmd(
        nc, in_maps, core_ids=list(range(CORES)), trace=trace)
    return res


def kernel(u_hat, routing_num):
    res = run(u_hat, routing_num, trace=False)
    return np.asarray(res.results[0]["v_out"], dtype=np.float32)


# revision 15
# speedup vs baseline: 1.4443x; 1.2444x over previous
"""DGL capsule routing layer on 8 trn2 NeuronCores (Bass/Tile).

Math: for routing_num iterations,
    c = softmax(b, axis=out)                        # b0 = 0
    s = einsum('io,iof->of', c, uh)
    v = squash(s)
    b = b + einsum('iof,of->io', uh, v)
Output: final v [OUT, F].

Key identity: b_t = uh . (v_1 + ... + v_t)  (b is linear in uh), so b is
never materialized across iterations; each iteration is one streaming pass
over uh with w_t = cumulative sum of v's:
    pass t: b = sum_f uh[i,o,f]*w[o,f]; e = exp(b); r_i = 1/sum_o e
            s[o,f] = sum_i r_i * e[i,o] * uh[i,o,f]   (partial per core)
            AllReduce(s); v = squash(s); w += v
Pass 1 has c uniform (=1/OUT) so it is a pure PE pass.

Sharding: i (in_nodes) split across 8 cores, 512 rows each (4 blocks of
128 partitions). Engine plan per 2048-wide o-f chunk (passes >= 2):
  GpSimd: tm = uh * w_bcast        (2-input mul; DVE TT never contends)
  DVE:    b-slice = segsum_f(tm);  p = e * uh (e broadcast over f)
  ACT:    e = exp(b) with fused denominator accum; psum flushes
  PE:     s-partial = sum_i rinv[i]*p[i,:] as 4x N=512 matmuls with
          rinv as the 1-column stationary operand -> psum [1,2048]
The per-block s partials go straight to DRAM [4,16384]; the AllReduce sums
over cores, and the cheap cross-block sum happens after the AR in the
partition-spread [128,128] layout (3 DVE adds).
"""

import numpy as np
from contextlib import ExitStack

import concourse.bass as bass
import concourse.mybir as mybir
import concourse.tile as tile
from concourse import bacc
from concourse import bass_utils

F32 = mybir.dt.float32
AX = mybir.AxisListType
AF = mybir.ActivationFunctionType

IN_NODES, OUT_NODES, F_SIZE = 4096, 1024, 16
CORES = 8
I_LOC = IN_NODES // CORES          # 512 in-nodes per core
ROW = OUT_NODES * F_SIZE           # 16384 floats per in-node row
P = 128
NBLK = I_LOC // P                  # 4 i-blocks per core
QT = 4096                          # streamed quarter width (elems/partition)
NQT = ROW // QT                    # 4 quarters per block
CH = 2048                          # chunk/piece width (elems/partition)
NCH_Q = QT // CH                   # 2 chunks per quarter
NMM = CH // 512                    # 4 matmuls per piece
F32R_MM = True                     # fast-path fp32 matmuls (1 cyc/row)
MM_DT = mybir.dt.float32r if F32R_MM else F32


def _body(nc, tc, uh, v_out, R, rg):
    uh_t = uh.rearrange("(n p) r -> n p r", p=P)   # [NBLK, 128, 16384]

    with ExitStack() as ctx:
        io = ctx.enter_context(tc.tile_pool(name="io", bufs=5))
        work = ctx.enter_context(tc.tile_pool(name="work", bufs=3))
        small = ctx.enter_context(tc.tile_pool(name="small", bufs=2))
        persist = ctx.enter_context(tc.tile_pool(name="persist", bufs=1))
        pspool = ctx.enter_context(tc.tile_pool(name="pspool", bufs=2, space="PSUM"))
        dram = ctx.enter_context(tc.tile_pool(name="dram", bufs=2, space="DRAM"))

        c0_f = persist.tile([P, 1], F32, name="c0_f")
        nc.vector.memset(c0_f, 1.0 / OUT_NODES)
        c0 = persist.tile([P, 1], MM_DT, name="c0")
        nc.vector.tensor_copy(c0, c0_f)
        w_sb = w_acc = None
        if R > 1:
            w_sb = persist.tile([P, ROW], F32, name="w_sb")
            w_acc = persist.tile([P, P], F32, name="w_acc")

        for t in range(1, R + 1):
            ar_in = dram.tile([NBLK, ROW], F32, tag="ar_in")
            for blk in range(NBLK):
                uts = []
                for q in range(NQT):
                    ut = io.tile([P, QT], F32, tag="ut")
                    nc.sync.dma_start(ut, uh_t[blk, :, q * QT:(q + 1) * QT])
                    uts.append(ut)
                if t == 1:
                    rinv = c0
                else:
                    b = small.tile([P, OUT_NODES], F32, tag="b")
                    for q in range(NQT):
                        for k in range(NCH_Q):
                            sl = slice(k * CH, (k + 1) * CH)
                            g0 = q * QT + k * CH
                            tm = work.tile([P, CH], F32, tag="tm")
                            # b-mul on GpSimd (concurrent with DVE TT/reduce)
                            nc.gpsimd.tensor_mul(
                                tm, uts[q][:, sl], w_sb[:, g0:g0 + CH])
                            o0 = g0 // F_SIZE
                            nc.vector.reduce_sum(
                                b[:, o0:o0 + CH // F_SIZE],
                                tm.rearrange("p (o f) -> p o f", f=F_SIZE),
                                axis=AX.X,
                            )
                    e = small.tile([P, OUT_NODES], F32, tag="e")
                    den = small.tile([P, 1], F32, tag="den")
                    nc.scalar.activation(e, b, AF.Exp, accum_out=den)
                    rinv_f = small.tile([P, 1], F32, tag="rinv_f")
                    nc.vector.reciprocal(rinv_f, den)
                    rinv = small.tile([P, 1], MM_DT, tag="rinv")
                    nc.vector.tensor_copy(rinv, rinv_f)
                for q in range(NQT):
                    for k in range(NCH_Q):
                        sl = slice(k * CH, (k + 1) * CH)
                        g0 = q * QT + k * CH
                        pt = work.tile([P, CH], MM_DT, tag="tm")
                        if t == 1:
                            # round to f32r on idle DVE (pass 1 only)
                            nc.vector.tensor_copy(pt, uts[q][:, sl])
                        else:
                            o0 = g0 // F_SIZE
                            och = CH // F_SIZE
                            nc.vector.tensor_mul(
                                pt.rearrange("p (o f) -> p o f", f=F_SIZE),
                                uts[q][:, sl].rearrange(
                                    "p (o f) -> p o f", f=F_SIZE),
                                e[:, o0:o0 + och][:, :, None].broadcast_to(
                                    [P, och, F_SIZE]),
                            )
                        ps = pspool.tile([1, CH], F32, tag="ps")
                        for c in range(NMM):
                            nc.tensor.matmul(
                                ps[:, c * 512:(c + 1) * 512],
                                rinv,
                                pt[:, c * 512:(c + 1) * 512],
                                start=True, stop=True,
                                skip_group_check=True,
                            )
                        fl = small.tile([1, CH], F32, tag="fl")
                        nc.scalar.copy(fl, ps)
                        nc.sync.dma_start(ar_in[blk, g0:g0 + CH], fl)
            ar_out = dram.tile([NBLK, ROW], F32, tag="ar_out")
            nc.gpsimd.collective_compute(
                "AllReduce", mybir.AluOpType.add, replica_groups=rg,
                ins=[ar_in.opt()], outs=[ar_out.opt()],
            )
            # s2[p,(j,f)] with o = p*8+j: sum the 4 block rows post-AR
            slds = []
            for blk in range(NBLK):
                sld = small.tile([P, P], F32, tag="sld", bufs=4)
                nc.sync.dma_start(
                    sld, ar_out[blk].rearrange("(p q) -> p q", p=P))
                slds.append(sld)
            s2 = small.tile([P, P], F32, tag="s2")
            nc.vector.tensor_add(s2, slds[0], slds[1])
            nc.vector.tensor_add(s2, s2, slds[2])
            nc.vector.tensor_add(s2, s2, slds[3])
            # squash: v = s * sqrt(sq)/(1+sq), sq = sum_f s^2
            ssq = small.tile([P, P], F32, tag="ssq")
            nc.vector.tensor_mul(ssq, s2, s2)
            sq = small.tile([P, 8], F32, tag="sq")
            nc.vector.reduce_sum(
                sq, ssq.rearrange("p (j f) -> p j f", f=F_SIZE), axis=AX.X)
            y = small.tile([P, 8], F32, tag="y")
            nc.scalar.sqrt(y, sq)
            # one Newton step: y <- 0.5*(y + sq/y) (ACT sqrt table is loose)
            ry = small.tile([P, 8], F32, tag="ry")
            nc.vector.reciprocal(ry, y)
            t1 = small.tile([P, 8], F32, tag="t1")
            nc.vector.tensor_mul(t1, sq, ry)
            nc.vector.tensor_add(t1, t1, y)
            nc.vector.tensor_scalar_mul(t1, t1, 0.5)
            d1 = small.tile([P, 8], F32, tag="d1")
            nc.vector.tensor_scalar_add(d1, sq, 1.0)
            rd = small.tile([P, 8], F32, tag="rd")
            nc.vector.reciprocal(rd, d1)
            sc = small.tile([P, 8], F32, tag="sc")
            nc.vector.tensor_mul(sc, t1, rd)
            v_sb = small.tile([P, P], F32, tag="v_sb")
            nc.vector.tensor_mul(
                v_sb.rearrange("p (j f) -> p j f", f=F_SIZE),
                s2.rearrange("p (j f) -> p j f", f=F_SIZE),
                sc[:, :, None].broadcast_to([P, 8, F_SIZE]),
            )
            if t == R:
                nc.sync.dma_start(
                    v_out.rearrange("(p j) f -> p (j f)", j=8), v_sb)
            else:
                if t == 1:
                    nc.scalar.copy(w_acc, v_sb)
                else:
                    nc.vector.tensor_add(w_acc, w_acc, v_sb)
                # broadcast w to all partitions via DRAM round-trip:
                # w_acc[p,(j,f)] -> flat w_dram[o*16+f] -> [128, ROW] bcast
                w_dram = dram.tile([ROW], F32, tag="w_dram")
                nc.sync.dma_start(
                    w_dram.rearrange("(p q) -> p q", p=P), w_acc)
                wd_b = w_dram.unsqueeze(0)
                for j in range(8):
                    sl = slice(j * CH, (j + 1) * CH)
                    nc.sync.dma_start(
                        w_sb[:, sl],
                        wd_b[:, sl].broadcast_to([P, CH]))


def _build(routing_num: int):
    R = int(routing_num)
    assert R >= 1
    nc = bacc.Bacc(
        "TRN2", target_bir_lowering=False, debug=False, num_devices=CORES)
    uh = nc.dram_tensor("uh", [I_LOC, ROW], F32, kind="ExternalInput")
    v_out = nc.dram_tensor("v_out", [OUT_NODES, F_SIZE], F32,
                           kind="ExternalOutput")
    rg = [list(range(CORES))]
    with tile.TileContext(nc) as tc:
        _body(nc, tc, uh.ap(), v_out.ap(), R, rg)
    nc.compile()
    return nc


_CACHE: dict = {}


def _get_nc(routing_num: int):
    R = int(routing_num)
    if R not in _CACHE:
        _CACHE[R] = _build(R)
    return _CACHE[R]


def _shard(u_hat: np.ndarray):
    uh = np.ascontiguousarray(np.asarray(u_hat, dtype=np.float32))
    assert uh.shape == (IN_NODES * OUT_NODES, F_SIZE), uh.shape
    uh = uh.reshape(IN_NODES, ROW)
    return [
        {"uh": np.ascontiguousarray(uh[k * I_LOC:(k + 1) * I_LOC])}
        for k in range(CORES)
    ]


def run(u_hat, routing_num, trace=False):
    nc = _get_nc(routing_num)
    in_maps = _shard(u_hat)
    res = bass_utils.run_bass_kernel_spmd(
        nc, in_maps, core_ids=list(range(CORES)), trace=trace)
    return res


def kernel(u_hat, routing_num):
    res = run(u_hat, routing_num, trace=False)
    return np.asarray(res.results[0]["v_out"], dtype=np.float32)
